# revision 15
# baseline (speedup 1.0000x reference)
"""GQA attention block (B=2, S=2048, DIM=4096, 32 Q heads / 8 KV heads, HD=128,
RoPE + causal softmax + output projection) on 8 trn2 NeuronCores.

Sharding: 8 cores = 2 batches x 4 head-groups. Core c handles batch c%2 and
head-group c//2 (8 Q heads, 2 KV heads). Each core computes a full-size
[S, DIM] partial of the output projection (its heads' contribution); the host
sums the 4 group-partials per batch.

Per-core kernel runs in transposed layout end to end. bf16 everywhere on the
matmul datapath (PSUM accumulation is fp32, so projections over the full
4096-dim contraction only pay input-quantization error, ~0.4%):
  - bf16 halves HBM traffic and SBUF footprint; the full wq (65 KB/part) and
    wo stay resident, so the Q projection is a single PSUM accumulation pass
    (no 2-level group accumulation) and the output projection streams no
    weights at all,
  - 3 DMA queues: sync (x tiles, wo prefetch, output writes), scalar
    (weight/table streams), gpsimd SWDGE (RoPE partition-half swaps),
  - bf16 runs 1 col/cycle at ANY free size, enabling exact 128-column causal
    trimming of scores/exp/PV/denominator work (62.5% -> 53.1% of full),
  - scores tiles in 4 rotating PSUM banks to hide the scores->exp->PV
    cross-engine round trip; exp output bf16; softmax denominators from a
    ones-column matmul accumulated in PSUM alongside PV,
  - RoPE is applied out of fp32 staging (PSUM copy) with a single bf16
    rounding on the final add; tables hold the 1/sqrt(hd) scale for Q,
  - output-projection PSUM->SBUF copies alternate Scalar/Vector engines.
"""

import math
import os
import sys
from contextlib import ExitStack
from dataclasses import dataclass

import numpy as np

sys.path.insert(0, "/opt/trn_rl_repo")

import concourse.bass as bass  # noqa: E402
import concourse.mybir as mybir  # noqa: E402
import concourse.tile as tile  # noqa: E402
from concourse import bacc  # noqa: E402

F32 = mybir.dt.float32
F32R = mybir.dt.float32r
BF16 = mybir.dt.bfloat16
P = 128


@dataclass(frozen=True)
class Cfg:
    S: int = 2048      # sequence length
    DIM: int = 4096    # model dim (contraction for projections)
    NH_L: int = 8      # q heads per core
    NKV_L: int = 2     # kv heads per core
    HD: int = 128      # head dim (must be P)
    TQ: int = 512      # token/query chunk (PSUM free dim)

    @property
    def CCH(self):  # contraction chunks
        return self.DIM // P

    @property
    def NT(self):  # token chunks
        return self.S // self.TQ

    @property
    def NKT(self):  # key tiles
        return self.S // P

    @property
    def RT(self):  # key tiles per token chunk
        return self.TQ // P

    @property
    def NREP(self):
        return self.NH_L // self.NKV_L


def build_program(cfg: Cfg) -> bass.Bass:
    nc = bacc.Bacc("TRN2", target_bir_lowering=False)
    S, DIM, NH_L, NKV_L, HD, TQ = cfg.S, cfg.DIM, cfg.NH_L, cfg.NKV_L, cfg.HD, cfg.TQ
    CCH, NT, RT = cfg.CCH, cfg.NT, cfg.RT
    MULT = mybir.AluOpType.mult
    EXP = mybir.ActivationFunctionType.Exp

    xT_d = nc.dram_tensor("xT", [DIM, S], BF16, kind="ExternalInput")
    wq_d = nc.dram_tensor("wq", [DIM, NH_L * HD], BF16, kind="ExternalInput")
    wk_d = nc.dram_tensor("wk", [DIM, NKV_L * HD], BF16, kind="ExternalInput")
    wv_d = nc.dram_tensor("wv", [DIM, NKV_L * HD], BF16, kind="ExternalInput")
    wo_d = nc.dram_tensor("wo", [NH_L * HD, DIM], BF16, kind="ExternalInput")
    cosq_d = nc.dram_tensor("cosq", [P, S], F32, kind="ExternalInput")
    sinq_d = nc.dram_tensor("sinq", [P, S], F32, kind="ExternalInput")
    cosk_d = nc.dram_tensor("cosk", [P, S], F32, kind="ExternalInput")
    sink_d = nc.dram_tensor("sink", [P, S], F32, kind="ExternalInput")
    maskT_d = nc.dram_tensor("maskT", [P, P], F32, kind="ExternalInput")
    out_d = nc.dram_tensor("out", [S, DIM], F32, kind="ExternalOutput")

    xT_r = xT_d.ap().rearrange("(co ci) t -> ci co t", ci=P)
    wq_r = wq_d.ap().rearrange("(co ci) d -> ci co d", ci=P)
    wk_r = wk_d.ap().rearrange("(co ci) d -> ci co d", ci=P)
    wv_r = wv_d.ap().rearrange("(co ci) d -> ci co d", ci=P)
    wo_r = wo_d.ap().rearrange("(dc p) m -> p dc m", p=P)

    def mm(out, lhsT, rhs, start, stop):
        nc.tensor.matmul(out, lhsT, rhs, start=start, stop=stop)

    with tile.TileContext(nc) as tc, ExitStack() as top:
        const = top.enter_context(tc.tile_pool(name="const", bufs=1))
        maskT_sb = const.tile([P, P], F32)
        nc.sync.dma_start(maskT_sb[:], maskT_d.ap())
        scratch_one = const.tile([P, 1], F32)
        nc.gpsimd.memset(scratch_one[:], 1.0)
        ones_col = const.tile([P, 1], BF16)
        nc.vector.tensor_copy(ones_col[:], scratch_one[:])

        kvp = top.enter_context(tc.tile_pool(name="kvp", bufs=1))
        KT_sb = kvp.tile([P, NKV_L, S], BF16)
        V_sb = kvp.tile([P, cfg.NKT, NKV_L * HD], BF16)

        # Q-projection weights + rope tables: pools created before phase A so
        # their DMAs (emitted inside phase A, after A's own weights, scalar
        # queue) stream during phase A's compute. Released after phase Q.
        qtab = tc.alloc_tile_pool(name="qtab", bufs=1)
        cosq_sb = qtab.tile([P, S], F32)
        sinq_sb = qtab.tile([P, S], F32)
        wqf = tc.alloc_tile_pool(name="wqf", bufs=1)
        wq_sb = wqf.tile([P, CCH, NH_L * HD], BF16)

        # ---------------- Phase A: K^T and V projections (+ RoPE on K) -----
        with ExitStack() as ctx:
            wkvp = ctx.enter_context(tc.tile_pool(name="wkvp", bufs=1))
            ktab = ctx.enter_context(tc.tile_pool(name="ktab", bufs=1))
            xap = ctx.enter_context(tc.tile_pool(name="xap", bufs=8))
            rtp = ctx.enter_context(tc.tile_pool(name="rtp", bufs=8))
            pka = ctx.enter_context(tc.tile_pool(name="pka", bufs=2, space="PSUM"))
            pva = ctx.enter_context(tc.tile_pool(name="pva", bufs=1, space="PSUM"))

            wk_sb = wkvp.tile([P, CCH, NKV_L * HD], BF16)
            wv_sb = wkvp.tile([P, CCH, NKV_L * HD], BF16)
            for i in range(0, CCH, 8):
                nc.scalar.dma_start(wk_sb[:, i:i + 8, :], wk_r[:, i:i + 8, :])
                nc.scalar.dma_start(wv_sb[:, i:i + 8, :], wv_r[:, i:i + 8, :])
            cosk_sb = ktab.tile([P, S], F32)
            nc.scalar.dma_start(cosk_sb[:], cosk_d.ap())
            sink_sb = ktab.tile([P, S], F32)
            nc.scalar.dma_start(sink_sb[:], sink_d.ap())
            # phase-Q table prefetch; wq chunks are spread across the tn loop
            # below so the x stream keeps bandwidth priority early in A.
            nc.scalar.dma_start(cosq_sb[:], cosq_d.ap())
            nc.scalar.dma_start(sinq_sb[:], sinq_d.ap())

            for tn in range(NT):
                tsl = slice(tn * TQ, (tn + 1) * TQ)
                psk = [pka.tile([P, TQ], F32, tag=f"psk{d}", name=f"psk{d}")
                       for d in range(NKV_L)]
                psv = [pva.tile([P, NKV_L * HD], F32, tag=f"psv{j}",
                                name=f"psv{j}") for j in range(RT)]
                for c0 in range(0, CCH, 2):
                    xt2 = xap.tile([P, 2, TQ], BF16, tag="xa", name="xa")
                    nc.sync.dma_start(xt2[:], xT_r[:, c0:c0 + 2, tsl])
                    for ci in range(2):
                        c = c0 + ci
                        xt = xt2[:, ci, :]
                        st, sp = c == 0, c == CCH - 1
                        for d in range(NKV_L):
                            mm(psk[d][:], wk_sb[:, c, d * HD:(d + 1) * HD],
                               xt, st, sp)
                        for j in range(RT):
                            mm(psv[j][:], xt2[:, ci, j * P:(j + 1) * P],
                               wv_sb[:, c, :], st, sp)
                # spread over tn 1..3 so x has full DMA bandwidth in tn 0
                nwq = [0, 12, 12, 8][tn]
                wq0 = [0, 0, 12, 24][tn]
                for i in range(wq0, wq0 + nwq, 4):
                    nc.scalar.dma_start(wq_sb[:, i:i + 4, :],
                                        wq_r[:, i:i + 4, :])
                for j in range(RT):
                    nc.scalar.copy(V_sb[:, tn * RT + j, :], psv[j][:])
                for d in range(NKV_L):
                    kst = rtp.tile([P, TQ], F32, tag="kst", name="kst")
                    nc.scalar.copy(kst[:], psk[d][:])
                    t = rtp.tile([P, TQ], F32, tag="ktmp", name="ktmp")
                    nc.gpsimd.dma_start(t[0:64], kst[64:128])
                    nc.gpsimd.dma_start(t[64:128], kst[0:64])
                    nc.vector.tensor_tensor(t[:], t[:], sink_sb[:, tsl], MULT)
                    nc.vector.tensor_tensor(KT_sb[:, d, tsl], kst[:],
                                            cosk_sb[:, tsl], MULT)
                    nc.vector.tensor_add(KT_sb[:, d, tsl], KT_sb[:, d, tsl],
                                         t[:])

        # ---------------- Phase Q: Q^T projection (single PSUM pass) -------
        qtbp = tc.alloc_tile_pool(name="qtbp", bufs=1, side="right")
        qtb_sb = qtbp.tile([P, NH_L, S], BF16)
        with ExitStack() as ctx:
            xap = ctx.enter_context(tc.tile_pool(name="xqp", bufs=8))
            rtp = ctx.enter_context(tc.tile_pool(name="rtq", bufs=8))
            pqa = ctx.enter_context(tc.tile_pool(name="pqa", bufs=1, space="PSUM"))

            for tn in range(NT):
                tsl = slice(tn * TQ, (tn + 1) * TQ)
                psq = [pqa.tile([P, TQ], F32, tag=f"psq{h}", name=f"psq{h}")
                       for h in range(NH_L)]
                for c0 in range(0, CCH, 2):
                    xt2 = xap.tile([P, 2, TQ], BF16, tag="xq", name="xq")
                    nc.sync.dma_start(xt2[:], xT_r[:, c0:c0 + 2, tsl])
                    for ci in range(2):
                        c = c0 + ci
                        st, sp = c == 0, c == CCH - 1
                        for h in range(NH_L):
                            mm(psq[h][:], wq_sb[:, c, h * HD:(h + 1) * HD],
                               xt2[:, ci, :], st, sp)
                for h in range(NH_L):
                    qs = rtp.tile([P, TQ], F32, tag="qst", name="qst")
                    nc.scalar.copy(qs[:], psq[h][:])
                    t = rtp.tile([P, TQ], F32, tag="qtmp", name="qtmp")
                    nc.gpsimd.dma_start(t[0:64], qs[64:128])
                    nc.gpsimd.dma_start(t[64:128], qs[0:64])
                    nc.vector.tensor_tensor(t[:], t[:], sinq_sb[:, tsl], MULT)
                    nc.vector.tensor_tensor(qtb_sb[:, h, tsl], qs[:],
                                            cosq_sb[:, tsl], MULT)
                    nc.vector.tensor_add(qtb_sb[:, h, tsl], qtb_sb[:, h, tsl],
                                         t[:])

        wqf.release()
        qtab.release()

        # ---------------- Phase S: attention per head ----------------------
        atp = tc.alloc_tile_pool(name="atp", bufs=1, side="right")
        attnT_sb = atp.tile([P, NH_L, S], BF16)
        wop = tc.alloc_tile_pool(name="wop", bufs=1, side="right")
        wo_sb = wop.tile([P, NH_L, DIM], BF16)
        for i in range(NH_L):
            nc.sync.dma_start(wo_sb[:, i, :], wo_r[:, i, :])

        with ExitStack() as ctx:
            ptp = ctx.enter_context(tc.tile_pool(name="ptp", bufs=4))
            uop = ctx.enter_context(tc.tile_pool(name="uop", bufs=3))
            bcp = ctx.enter_context(tc.tile_pool(name="bcp", bufs=2))
            psc = ctx.enter_context(tc.tile_pool(name="psc", bufs=3, space="PSUM"))
            pso = ctx.enter_context(tc.tile_pool(name="pso", bufs=1, space="PSUM"))
            pss = ctx.enter_context(tc.tile_pool(name="pss", bufs=1, space="PSUM"))

            # Key tiles are processed in pairs: two 512-col score tiles land
            # in one 2-bank [128, 1024] PSUM tile and ONE ACT op exps both
            # (halving the ACT op count, which gates this phase). For the
            # diagonal pairs the unused gap region is exp'd as garbage but
            # never read back (PV/ones matmuls stick to the valid regions).
            for qc in range(NT):
                qbase = qc * TQ
                qsl = slice(qbase, qbase + TQ)
                nkt = (qc + 1) * RT
                for h in range(NH_L):
                    g = h // cfg.NREP
                    ps_out = pso.tile([P, TQ], F32, tag="psout", name="psout")
                    ps_sum = pss.tile([1, TQ], F32, tag="pssum", name="pssum")
                    for pr in range(nkt // 2):
                        ktA, ktB = 2 * pr, 2 * pr + 1
                        diag = ktA >= qc * RT
                        q0A = (ktA - qc * RT) * P if diag else 0
                        q0B = q0A + P if diag else 0
                        psc2 = psc.tile([P, 2 * TQ], F32, tag="psc2",
                                        name="psc2")
                        pt2 = ptp.tile([P, 2 * TQ], BF16, tag="pt2",
                                       name="pt2")
                        mm(psc2[:, q0A:TQ],
                           KT_sb[:, g, ktA * P:(ktA + 1) * P],
                           qtb_sb[:, h, qbase + q0A:qbase + TQ], True, True)
                        mm(psc2[:, TQ + q0B:2 * TQ],
                           KT_sb[:, g, ktB * P:(ktB + 1) * P],
                           qtb_sb[:, h, qbase + q0B:qbase + TQ], True, True)
                        if diag:
                            nc.vector.tensor_add(psc2[:, q0A:q0A + P],
                                                 psc2[:, q0A:q0A + P],
                                                 maskT_sb[:])
                            nc.vector.tensor_add(
                                psc2[:, TQ + q0B:TQ + q0B + P],
                                psc2[:, TQ + q0B:TQ + q0B + P], maskT_sb[:])
                        if q0A == 0:
                            nc.scalar.activation(pt2[:, 0:2 * TQ],
                                                 psc2[:, 0:2 * TQ], EXP)
                        else:
                            nc.scalar.activation(pt2[:, q0A:TQ],
                                                 psc2[:, q0A:TQ], EXP)
                            nc.scalar.activation(pt2[:, TQ + q0B:2 * TQ],
                                                 psc2[:, TQ + q0B:2 * TQ],
                                                 EXP)
                        first, last = pr == 0, pr == nkt // 2 - 1
                        vA = V_sb[:, ktA, g * HD:(g + 1) * HD]
                        vB = V_sb[:, ktB, g * HD:(g + 1) * HD]
                        mm(ps_out[:, q0A:TQ], vA, pt2[:, q0A:TQ], first,
                           False)
                        mm(ps_out[:, q0B:TQ], vB, pt2[:, TQ + q0B:2 * TQ],
                           False, last)
                        mm(ps_sum[:, q0A:TQ], ones_col[:], pt2[:, q0A:TQ],
                           first, False)
                        mm(ps_sum[:, q0B:TQ], ones_col[:],
                           pt2[:, TQ + q0B:2 * TQ], False, last)
                    # Free the single PSUM banks immediately: dump the
                    # unnormalized PV output to SBUF (bf16) and take the
                    # reciprocal, so the next head's PV/ones can start while
                    # the normalize chain (broadcast+mult, SBUF-only) trails.
                    uo = uop.tile([P, TQ], BF16, tag="uo", name="uo")
                    nc.vector.tensor_copy(uo[:], ps_out[:])
                    rrow = bcp.tile([1, TQ], F32, tag="rrow", name="rrow")
                    nc.vector.reciprocal_approx_fast(out=rrow[:], in_=ps_sum[:])
                    bc_sb = bcp.tile([P, TQ], F32, tag="bcsb", name="bcsb")
                    nc.gpsimd.partition_broadcast(bc_sb[:], rrow[:])
                    nc.vector.tensor_tensor(attnT_sb[:, h, qsl], uo[:],
                                            bc_sb[:], MULT)

        # ---------------- Phase W: output projection -----------------------
        with ExitStack() as ctx:
            owp = ctx.enter_context(tc.tile_pool(name="owp", bufs=4))
            psw = ctx.enter_context(tc.tile_pool(name="psw", bufs=4, space="PSUM"))

            bi = 0
            for mc in range(DIM // TQ):
                msl = slice(mc * TQ, (mc + 1) * TQ)
                for tb in range(S // P):
                    ps_w = psw.tile([P, TQ], F32, tag="psw", name="psw")
                    for dc in range(NH_L):
                        mm(ps_w[:], attnT_sb[:, dc, tb * P:(tb + 1) * P],
                           wo_sb[:, dc, msl], dc == 0, dc == NH_L - 1)
                    ot = owp.tile([P, TQ], F32, tag="ot", name="ot")
                    if bi % 2 == 0:
                        nc.scalar.copy(ot[:], ps_w[:])
                    else:
                        nc.vector.tensor_copy(ot[:], ps_w[:])
                    bi += 1
                    nc.sync.dma_start(out_d.ap()[tb * P:(tb + 1) * P, msl],
                                      ot[:])

        wop.release()
        atp.release()
        qtbp.release()

    nc.compile()
    return nc


# ---------------------------------------------------------------------------
# Host side
# ---------------------------------------------------------------------------

_HALF_PERM = np.concatenate([np.arange(0, P, 2), np.arange(1, P, 2)])

LAST_EXEC_NS = None
LAST_RESULTS = None


def _host_prep(cfg: Cfg, x, wq, wk, wv, wo, freqs_cos, freqs_sin):
    """Build the 8 per-core input maps. Core c: batch c % 2, group c // 2."""
    import ml_dtypes

    BF = ml_dtypes.bfloat16
    B = x.shape[0]
    n_groups = wq.shape[1] // (cfg.NH_L * cfg.HD)
    hd = cfg.HD

    cosT = np.ascontiguousarray(freqs_cos.T.astype(np.float32))  # [HD/2, S]
    sinT = np.ascontiguousarray(freqs_sin.T.astype(np.float32))
    sc = np.float32(1.0 / math.sqrt(hd))
    cosq = np.concatenate([cosT, cosT], 0) * sc
    sinq = np.concatenate([-sinT, sinT], 0) * sc
    cosk = np.concatenate([cosT, cosT], 0)
    sink = np.concatenate([-sinT, sinT], 0)
    maskT = np.tril(np.full((P, P), -1e9, np.float32), -1)

    xT = [np.ascontiguousarray(x[b].T).astype(BF) for b in range(B)]

    def permute_cols(w, nheads):
        w = w.reshape(cfg.DIM, nheads, hd)[:, :, _HALF_PERM]
        return np.ascontiguousarray(w.reshape(cfg.DIM, nheads * hd)).astype(BF)

    in_maps = []
    qcols = cfg.NH_L * hd
    kcols = cfg.NKV_L * hd
    for c in range(B * n_groups):
        b, g = c % B, c // B
        in_maps.append(dict(
            xT=xT[b],
            wq=permute_cols(wq[:, g * qcols:(g + 1) * qcols], cfg.NH_L),
            wk=permute_cols(wk[:, g * kcols:(g + 1) * kcols], cfg.NKV_L),
            wv=np.ascontiguousarray(wv[:, g * kcols:(g + 1) * kcols]).astype(BF),
            wo=np.ascontiguousarray(wo[g * qcols:(g + 1) * qcols, :]).astype(BF),
            cosq=cosq, sinq=sinq, cosk=cosk, sink=sink, maskT=maskT,
        ))
    return in_maps


def kernel(x, wq, wk, wv, wo, freqs_cos, freqs_sin, mask, start_pos=0):
    global LAST_EXEC_NS, LAST_RESULTS
    x = np.asarray(x, np.float32)
    wq = np.asarray(wq, np.float32)
    wk = np.asarray(wk, np.float32)
    wv = np.asarray(wv, np.float32)
    wo = np.asarray(wo, np.float32)
    freqs_cos = np.asarray(freqs_cos, np.float32)
    freqs_sin = np.asarray(freqs_sin, np.float32)

    cfg = Cfg()
    B = x.shape[0]
    n_groups = 4
    in_maps = _host_prep(cfg, x, wq, wk, wv, wo, freqs_cos, freqs_sin)

    from concourse.bass_utils import run_bass_kernel_spmd

    nc = build_program(cfg)
    trace = bool(int(os.environ.get("KERNEL_TRACE", "0")))
    res = run_bass_kernel_spmd(nc, in_maps, core_ids=list(range(len(in_maps))),
                               trace=trace)
    LAST_EXEC_NS = res.exec_time_ns
    LAST_RESULTS = res

    out = np.zeros((B, cfg.S, cfg.DIM), np.float32)
    for c in range(B * n_groups):
        b = c % B
        out[b] += res.results[c]["out"]
    return out
